# revision 21
# baseline (speedup 1.0000x reference)
"""AttentionHead kernel for TRN2, data-parallel over batch across 8 NeuronCores.

Per core: one batch element.  Host passes xT (=x.T, bf16, window-major) so no
on-chip transposes of the input are needed.
  qkT[128, t] = [Wq*scale | Wk].T @ xT   (fused q+k projection; rows 0-63 = q,
                rows 64-127 = k)
  vT[64, t]   = Wv.T @ xT, PE-transposed to natural v'[k, 65] with a ones
                column (row 64 of the output = softmax denominator)
  sT[k, q]    = k @ q.T  + Toeplitz rel-pos bias (causal mask baked in); bias
                added either by DVE tensor_add or by PE identity-matmul
                accumulate (alternating, to balance engine load)
  attnT       = exp(sT)  (no max subtraction: |scores| < ~15 provably;
                exp of two k-chunks per ACT op to amortize the 352cy overhead)
  out'[65, q] = v'.T @ attnT    -> DMA'd out raw; host divides + transposes.

Bias strip: RT[p, j] = tb[j-2048-p] if 0 <= j-2048-p < 2048 else -30000;
bias tile for (k0, q0) is the slice RT[:, 2048+q0-k0 : +512].
"""

import numpy as np
import ml_dtypes
from contextlib import ExitStack

import concourse.bass as bass
import concourse.bacc as bacc
from concourse import mybir
from concourse.tile import TileContext
from concourse.masks import make_identity

B, T, C, H = 8, 2048, 768, 64
NCORES = 8
RTW = 4096  # bias strip width
MASK_NEG = -30000.0
BF = mybir.dt.bfloat16
F32 = mybir.dt.float32
AF = mybir.ActivationFunctionType

NT = T // 128   # 16 t-chunks
NCC = C // 128  # 6 c-chunks
NW = T // 512   # 4 q-windows


def build_nc():
    nc = bacc.Bacc()
    # x.T, window-major: [NW, C, 512]
    xt = nc.declare_dram_parameter("xt", [NW, C, 512], BF, isOutput=False)
    wqk = nc.declare_dram_parameter("wqk", [C, 2 * H], BF, isOutput=False)
    wv = nc.declare_dram_parameter("wv", [C, H], BF, isOutput=False)
    rt = nc.declare_dram_parameter("rt", [128, RTW], BF, isOutput=False)
    out = nc.declare_dram_parameter("out", [H + 1, T], F32, isOutput=True)

    with TileContext(nc) as tc, ExitStack() as ctx:
        _body(tc, ctx, xt, wqk, wv, rt, out)
    nc.compile()
    return nc


def _rt_pair_ap(rt_sb, j0, width):
    """AP over the bias strip shaped [128, 2, width]: bank b -> columns
    j0 - 128*b + f  (matching k-chunk pairs kc, kc+1)."""
    base = rt_sb[:, j0:j0 + width]
    return bass.AP(tensor=base.tensor, offset=base.offset,
                   ap=[base.ap[0], [-128, 2], base.ap[1]])


def _body(tc, ctx, xt, wqk, wv, rt, out):
    nc = tc.nc
    const = ctx.enter_context(tc.tile_pool(name="const", bufs=1))
    big = ctx.enter_context(tc.tile_pool(name="big", bufs=1))
    work = ctx.enter_context(tc.tile_pool(name="work", bufs=9))
    psum_mm = ctx.enter_context(tc.tile_pool(name="psum_mm", bufs=3, space="PSUM"))
    psum_o = ctx.enter_context(tc.tile_pool(name="psum_o", bufs=2, space="PSUM"))

    # ---- all input DMAs first; order = first-needed first (FIFO queue).
    # x.T arrives per-cc-chunk so the projection matmuls track DMA arrival.
    wqk_sb = const.tile([128, NCC, 2 * H], BF)
    nc.sync.dma_start(out=wqk_sb, in_=wqk.rearrange("(cc p) h -> p cc h", p=128))
    xTs = []
    for w in range(NW):
        xTw = big.tile([128, NCC, 512], BF, tag=f"xT{w}")
        xTs.append(xTw)

    def dma_xt(w):
        src = xt[w].rearrange("(cc p) t -> cc p t", p=128)
        for cc in range(NCC):
            nc.sync.dma_start(out=xTs[w][:, cc, :], in_=src[cc])

    dma_xt(0)
    wv_sb = const.tile([128, NCC, H], BF)
    nc.sync.dma_start(out=wv_sb, in_=wv.rearrange("(cc p) h -> p cc h", p=128))
    rt_sb = const.tile([128, RTW], BF)
    nc.sync.dma_start(out=rt_sb, in_=rt[:, :])
    for w in range(1, NW):
        dma_xt(w)

    id128 = const.tile([128, 128], BF)
    make_identity(nc, id128)

    qT = big.tile([64, T], BF)            # pre-scaled
    kT = big.tile([64, T], BF)
    vT = big.tile([64, T], BF)
    v_sb = big.tile([128, NT, H + 1], BF)
    nc.vector.memset(v_sb[:, :, H:H + 1], 1.0)

    state = {"pair_idx": 0}

    def emit_proj(w):
        qsl = slice(w * 512, (w + 1) * 512)
        xTw = xTs[w]
        pm = psum_mm.tile([128, 512], F32, tag="mm")
        for cc in range(NCC):
            nc.tensor.matmul(pm, lhsT=wqk_sb[:, cc, :], rhs=xTw[:, cc, :],
                             start=(cc == 0), stop=(cc == NCC - 1),
                             skip_group_check=True)
        nc.scalar.activation(out=qT[:, qsl], in_=pm[0:64, :], func=AF.Copy)
        nc.vector.tensor_copy(out=kT[:, qsl], in_=pm[64:128, :])
        pv = psum_mm.tile([64, 512], F32, tag="mm")
        for cc in range(NCC):
            nc.tensor.matmul(pv, lhsT=wv_sb[:, cc, :], rhs=xTw[:, cc, :],
                             start=(cc == 0), stop=(cc == NCC - 1),
                             skip_group_check=True)
        nc.vector.tensor_copy(out=vT[:, qsl], in_=pv)

    def emit_vtr(w):
        # 4 chunk-transposes into one PSUM bank, one strided evac copy
        pvt = psum_mm.tile([128, 4, H], BF, tag="mm")
        for i, it in enumerate(range(4 * w, 4 * w + 4)):
            nc.tensor.transpose(pvt[:, i, :], vT[:, it * 128:(it + 1) * 128],
                                id128[:64, :64])
        nc.vector.tensor_copy(out=v_sb[:, 4 * w:4 * w + 4, 0:H], in_=pvt)

    def emit_qk(w):
        # causal narrowing at pair granularity: columns below off are fully
        # masked for both banks of the pair.
        q0 = w * 512
        nkc = 4 * (w + 1)
        at_tiles = []
        for kp in range(nkc // 2):
            kc = 2 * kp
            off = max(0, kc * 128 - q0)
            wd = 512 - off
            ps2 = psum_mm.tile([128, 2, 512], F32, tag="mm")
            on_pe = (state["pair_idx"] % 3 == 2)
            for b in range(2):
                k0 = (kc + b) * 128
                nc.tensor.matmul(ps2[:, b, off:512],
                                 lhsT=kT[:, k0:k0 + 128],
                                 rhs=qT[:, q0 + off:q0 + 512],
                                 start=True, stop=not on_pe,
                                 skip_group_check=True)
            j0 = 2048 + q0 - kc * 128
            if on_pe:
                for b in range(2):
                    nc.tensor.matmul(ps2[:, b, off:512], lhsT=id128,
                                     rhs=rt_sb[:, j0 - 128 * b + off:j0 - 128 * b + 512],
                                     start=False, stop=True,
                                     skip_group_check=True)
            else:
                nc.vector.tensor_add(ps2[:, :, off:512], ps2[:, :, off:512],
                                     _rt_pair_ap(rt_sb, j0 + off, wd))
            at2 = work.tile([128, 2, 512], BF, tag="at")
            nc.scalar.activation(out=at2[:, :, off:512], in_=ps2[:, :, off:512],
                                 func=AF.Exp)
            at_tiles.append(at2)
            state["pair_idx"] += 1
        return at_tiles

    def emit_pv(w, at_tiles):
        q0 = w * 512
        nkc = 4 * (w + 1)
        po = psum_o.tile([H + 1, 512], F32, tag="po")
        for kp in range(nkc // 2):
            kc = 2 * kp
            at2 = at_tiles[kp]
            for b in range(2):
                bo = max(0, (kc + b) * 128 - q0)
                nc.tensor.matmul(po[:, bo:512], lhsT=v_sb[:, kc + b, :],
                                 rhs=at2[:, b, bo:512],
                                 start=(kc + b == 0), stop=(kc + b == nkc - 1),
                                 skip_group_check=True)
        ob = work.tile([H + 1, 512], F32, tag="ob")
        nc.vector.tensor_copy(out=ob, in_=po)
        nc.gpsimd.dma_start(out=out[:, q0:q0 + 512], in_=ob)

    # software-pipelined window loop: PE order per iteration is
    # QK_w -> proj_{w+1} -> PV_w -> vtr_{w+1}, so every PE instruction's
    # inputs were produced while PE was busy elsewhere.
    emit_proj(0)
    emit_vtr(0)
    for w in range(NW):
        at_tiles = emit_qk(w)
        if w + 1 < NW:
            emit_proj(w + 1)
            emit_vtr(w + 1)
        emit_pv(w, at_tiles)


def make_host_inputs(input_tensor, Wq, Wk, Wv, bias_table):
    x = np.asarray(input_tensor, dtype=np.float32)
    scale = 1.0 / np.sqrt(H)
    wqk = np.concatenate([np.asarray(Wq, dtype=np.float32) * scale,
                          np.asarray(Wk, dtype=np.float32)], axis=1)
    wqk_bf = np.ascontiguousarray(wqk.astype(ml_dtypes.bfloat16))
    wv_bf = np.ascontiguousarray(np.asarray(Wv, dtype=np.float32).astype(ml_dtypes.bfloat16))
    tb = np.asarray(bias_table, dtype=np.float32)[:, 0]
    p = np.arange(128)[:, None]
    j = np.arange(RTW)[None, :]
    idx = j - 2048 - p
    rtm = np.where((idx >= 0) & (idx < 2048), tb[np.clip(idx, 0, 2047)],
                   np.float32(MASK_NEG)).astype(ml_dtypes.bfloat16)
    rtm = np.ascontiguousarray(rtm)
    # per-core transposed bf16 input, window-major [NW, C, 512]
    xts = []
    for i in range(x.shape[0]):
        xt = x[i].T.astype(ml_dtypes.bfloat16)          # [C, T]
        xts.append(np.ascontiguousarray(
            xt.reshape(C, NW, 512).transpose(1, 0, 2)))  # [NW, C, 512]
    return xts, wqk_bf, wv_bf, rtm


def finish_host(raw):
    """raw: [65, T] f32 -> [T, H] f32 (divide by denominator row, transpose)."""
    return np.ascontiguousarray((raw[0:H, :] / raw[H:H + 1, :]).T)


_NC_CACHE = {}


def kernel(input_tensor, Wq, Wk, Wv, bias_table):
    from concourse.bass_utils import run_bass_kernel_spmd
    xts, wqk_bf, wv_bf, rtm = make_host_inputs(input_tensor, Wq, Wk, Wv, bias_table)
    if "nc" not in _NC_CACHE:
        _NC_CACHE["nc"] = build_nc()
    nc = _NC_CACHE["nc"]
    in_maps = [{"xt": xts[i], "wqk": wqk_bf, "wv": wv_bf, "rt": rtm}
               for i in range(NCORES)]
    res = run_bass_kernel_spmd(nc, in_maps, list(range(NCORES)))
    return np.stack([finish_host(np.asarray(res.results[i]["out"], dtype=np.float32))
                     for i in range(NCORES)], axis=0)


# revision 22
# speedup vs baseline: 1.0614x; 1.0614x over previous
"""AttentionHead kernel for TRN2, data-parallel over batch across 8 NeuronCores.

Per core: one batch element.  Host passes xT (=x.T, bf16, window-major) so no
on-chip transposes of the input are needed.
  qkT[128, t] = [Wq*scale | Wk].T @ xT   (fused q+k projection; rows 0-63 = q,
                rows 64-127 = k)
  vT[64, t]   = Wv.T @ xT, PE-transposed to natural v'[k, 65] with a ones
                column (row 64 of the output = softmax denominator)
  sT[k, q]    = k @ q.T  + Toeplitz rel-pos bias (causal mask baked in); bias
                added either by DVE tensor_add or by PE identity-matmul
                accumulate (alternating, to balance engine load)
  attnT       = exp(sT)  (no max subtraction: |scores| < ~15 provably;
                exp of two k-chunks per ACT op to amortize the 352cy overhead)
  out'[65, q] = v'.T @ attnT    -> DMA'd out raw; host divides + transposes.

Bias strip: RT[p, j] = tb[j-2048-p] if 0 <= j-2048-p < 2048 else -30000;
bias tile for (k0, q0) is the slice RT[:, 2048+q0-k0 : +512].
"""

import numpy as np
import ml_dtypes
from contextlib import ExitStack

import concourse.bass as bass
import concourse.bacc as bacc
from concourse import mybir
from concourse.tile import TileContext
from concourse.masks import make_identity

B, T, C, H = 8, 2048, 768, 64
NCORES = 8
RTW = 4096  # bias strip width
MASK_NEG = -30000.0
BF = mybir.dt.bfloat16
F32 = mybir.dt.float32
AF = mybir.ActivationFunctionType

NT = T // 128   # 16 t-chunks
NCC = C // 128  # 6 c-chunks
NW = T // 512   # 4 q-windows


def build_nc():
    nc = bacc.Bacc()
    # x.T, window-major: [NW, C, 512]
    xt = nc.declare_dram_parameter("xt", [NW, C, 512], BF, isOutput=False)
    wqk = nc.declare_dram_parameter("wqk", [C, 2 * H], BF, isOutput=False)
    wv = nc.declare_dram_parameter("wv", [C, H], BF, isOutput=False)
    rt = nc.declare_dram_parameter("rt", [128, RTW], BF, isOutput=False)
    out = nc.declare_dram_parameter("out", [H + 1, T], F32, isOutput=True)

    with TileContext(nc) as tc, ExitStack() as ctx:
        _body(tc, ctx, xt, wqk, wv, rt, out)
    nc.compile()
    return nc


def _rt_pair_ap(rt_sb, j0, width):
    """AP over the bias strip shaped [128, 2, width]: bank b -> columns
    j0 - 128*b + f  (matching k-chunk pairs kc, kc+1)."""
    base = rt_sb[:, j0:j0 + width]
    return bass.AP(tensor=base.tensor, offset=base.offset,
                   ap=[base.ap[0], [-128, 2], base.ap[1]])


def _body(tc, ctx, xt, wqk, wv, rt, out):
    nc = tc.nc
    const = ctx.enter_context(tc.tile_pool(name="const", bufs=1))
    big = ctx.enter_context(tc.tile_pool(name="big", bufs=1))
    work = ctx.enter_context(tc.tile_pool(name="work", bufs=9))
    psum_mm = ctx.enter_context(tc.tile_pool(name="psum_mm", bufs=3, space="PSUM"))
    psum_o = ctx.enter_context(tc.tile_pool(name="psum_o", bufs=2, space="PSUM"))

    # ---- all input DMAs first; order = first-needed first (FIFO queue).
    # x.T arrives per-cc-chunk so the projection matmuls track DMA arrival.
    wqk_sb = const.tile([128, NCC, 2 * H], BF)
    nc.sync.dma_start(out=wqk_sb, in_=wqk.rearrange("(cc p) h -> p cc h", p=128))
    xTs = []
    for w in range(NW):
        xTw = big.tile([128, NCC, 512], BF, tag=f"xT{w}")
        xTs.append(xTw)

    def dma_xt(w):
        src = xt[w].rearrange("(cc p) t -> cc p t", p=128)
        for cc in range(NCC):
            nc.sync.dma_start(out=xTs[w][:, cc, :], in_=src[cc])

    dma_xt(0)
    wv_sb = const.tile([128, NCC, H], BF)
    nc.sync.dma_start(out=wv_sb, in_=wv.rearrange("(cc p) h -> p cc h", p=128))
    rt_sb = const.tile([128, RTW], BF)
    nc.sync.dma_start(out=rt_sb, in_=rt[:, :])
    for w in range(1, NW):
        dma_xt(w)

    id128 = const.tile([128, 128], BF)
    make_identity(nc, id128)

    qT = big.tile([64, T], BF)            # pre-scaled
    kT = big.tile([64, T], BF)
    vT = big.tile([64, T], BF)
    v_sb = big.tile([128, NT, H + 1], BF)
    nc.vector.memset(v_sb[:, :, H:H + 1], 1.0)

    state = {"pair_idx": 0}

    def emit_proj(w):
        qsl = slice(w * 512, (w + 1) * 512)
        xTw = xTs[w]
        pm = psum_mm.tile([128, 512], F32, tag="mm")
        for cc in range(NCC):
            nc.tensor.matmul(pm, lhsT=wqk_sb[:, cc, :], rhs=xTw[:, cc, :],
                             start=(cc == 0), stop=(cc == NCC - 1),
                             skip_group_check=True)
        nc.scalar.activation(out=qT[:, qsl], in_=pm[0:64, :], func=AF.Copy)
        nc.vector.tensor_copy(out=kT[:, qsl], in_=pm[64:128, :])
        pv = psum_mm.tile([64, 512], F32, tag="mm")
        for cc in range(NCC):
            nc.tensor.matmul(pv, lhsT=wv_sb[:, cc, :], rhs=xTw[:, cc, :],
                             start=(cc == 0), stop=(cc == NCC - 1),
                             skip_group_check=True)
        nc.vector.tensor_copy(out=vT[:, qsl], in_=pv)

    def emit_vtr(w):
        # 4 chunk-transposes into one PSUM bank, one strided evac copy
        pvt = psum_mm.tile([128, 4, H], BF, tag="mm")
        for i, it in enumerate(range(4 * w, 4 * w + 4)):
            nc.tensor.transpose(pvt[:, i, :], vT[:, it * 128:(it + 1) * 128],
                                id128[:64, :64])
        nc.vector.tensor_copy(out=v_sb[:, 4 * w:4 * w + 4, 0:H], in_=pvt)

    def emit_qk(w):
        # causal narrowing at pair granularity: columns below off are fully
        # masked for both banks of the pair.
        q0 = w * 512
        nkc = 4 * (w + 1)
        at_tiles = []
        for kp in range(nkc // 2):
            kc = 2 * kp
            off = max(0, kc * 128 - q0)
            wd = 512 - off
            ps2 = psum_mm.tile([128, 2, 512], F32, tag="mm")
            on_pe = (state["pair_idx"] % 2 == 1)
            for b in range(2):
                k0 = (kc + b) * 128
                nc.tensor.matmul(ps2[:, b, off:512],
                                 lhsT=kT[:, k0:k0 + 128],
                                 rhs=qT[:, q0 + off:q0 + 512],
                                 start=True, stop=not on_pe,
                                 skip_group_check=True)
            j0 = 2048 + q0 - kc * 128
            if on_pe:
                for b in range(2):
                    nc.tensor.matmul(ps2[:, b, off:512], lhsT=id128,
                                     rhs=rt_sb[:, j0 - 128 * b + off:j0 - 128 * b + 512],
                                     start=False, stop=True,
                                     skip_group_check=True)
            else:
                nc.vector.tensor_add(ps2[:, :, off:512], ps2[:, :, off:512],
                                     _rt_pair_ap(rt_sb, j0 + off, wd))
            at2 = work.tile([128, 2, 512], BF, tag="at")
            nc.scalar.activation(out=at2[:, :, off:512], in_=ps2[:, :, off:512],
                                 func=AF.Exp)
            at_tiles.append(at2)
            state["pair_idx"] += 1
        return at_tiles

    def emit_pv(w, at_tiles):
        q0 = w * 512
        nkc = 4 * (w + 1)
        po = psum_o.tile([H + 1, 512], F32, tag="po")
        for kp in range(nkc // 2):
            kc = 2 * kp
            at2 = at_tiles[kp]
            for b in range(2):
                bo = max(0, (kc + b) * 128 - q0)
                nc.tensor.matmul(po[:, bo:512], lhsT=v_sb[:, kc + b, :],
                                 rhs=at2[:, b, bo:512],
                                 start=(kc + b == 0), stop=(kc + b == nkc - 1),
                                 skip_group_check=True)
        ob = work.tile([H + 1, 512], F32, tag="ob")
        nc.vector.tensor_copy(out=ob, in_=po)
        nc.gpsimd.dma_start(out=out[:, q0:q0 + 512], in_=ob)

    # software-pipelined window loop: PE order per iteration is
    # QK_w -> proj_{w+1} -> PV_w -> vtr_{w+1}, so every PE instruction's
    # inputs were produced while PE was busy elsewhere.
    emit_proj(0)
    emit_vtr(0)
    for w in range(NW):
        at_tiles = emit_qk(w)
        if w + 1 < NW:
            emit_proj(w + 1)
            emit_vtr(w + 1)
        emit_pv(w, at_tiles)


def make_host_inputs(input_tensor, Wq, Wk, Wv, bias_table):
    x = np.asarray(input_tensor, dtype=np.float32)
    scale = 1.0 / np.sqrt(H)
    wqk = np.concatenate([np.asarray(Wq, dtype=np.float32) * scale,
                          np.asarray(Wk, dtype=np.float32)], axis=1)
    wqk_bf = np.ascontiguousarray(wqk.astype(ml_dtypes.bfloat16))
    wv_bf = np.ascontiguousarray(np.asarray(Wv, dtype=np.float32).astype(ml_dtypes.bfloat16))
    tb = np.asarray(bias_table, dtype=np.float32)[:, 0]
    p = np.arange(128)[:, None]
    j = np.arange(RTW)[None, :]
    idx = j - 2048 - p
    rtm = np.where((idx >= 0) & (idx < 2048), tb[np.clip(idx, 0, 2047)],
                   np.float32(MASK_NEG)).astype(ml_dtypes.bfloat16)
    rtm = np.ascontiguousarray(rtm)
    # per-core transposed bf16 input, window-major [NW, C, 512]
    xts = []
    for i in range(x.shape[0]):
        xt = x[i].T.astype(ml_dtypes.bfloat16)          # [C, T]
        xts.append(np.ascontiguousarray(
            xt.reshape(C, NW, 512).transpose(1, 0, 2)))  # [NW, C, 512]
    return xts, wqk_bf, wv_bf, rtm


def finish_host(raw):
    """raw: [65, T] f32 -> [T, H] f32 (divide by denominator row, transpose)."""
    return np.ascontiguousarray((raw[0:H, :] / raw[H:H + 1, :]).T)


_NC_CACHE = {}


def kernel(input_tensor, Wq, Wk, Wv, bias_table):
    from concourse.bass_utils import run_bass_kernel_spmd
    xts, wqk_bf, wv_bf, rtm = make_host_inputs(input_tensor, Wq, Wk, Wv, bias_table)
    if "nc" not in _NC_CACHE:
        _NC_CACHE["nc"] = build_nc()
    nc = _NC_CACHE["nc"]
    in_maps = [{"xt": xts[i], "wqk": wqk_bf, "wv": wv_bf, "rt": rtm}
               for i in range(NCORES)]
    res = run_bass_kernel_spmd(nc, in_maps, list(range(NCORES)))
    return np.stack([finish_host(np.asarray(res.results[i]["out"], dtype=np.float32))
                     for i in range(NCORES)], axis=0)
